# revision 52
# baseline (speedup 1.0000x reference)
"""Bass/Trainium2 kernel for a 12-head self-attention block
(B=8, T=1024, C=768), data-parallel across 8 NeuronCores (one batch
element per core).

Per-core computation (batch element b):
  qkv   = x @ W_attn + b_attn            [T, 3C]
  scoresT[k, q] = k_h . q_h / 8 (+ mask bias), keys on partitions
  e     = exp(scoresT)                   (unnormalized)
  out_h = (v_h.T @ e_h) / (sum_k e_h)
  y     = concat(out_h) @ W_proj + b_proj

v6 design (all matmul operands bf16, fp32 PSUM accumulation):
  - attention is a uniform 12-unit (6 head-pairs x 2 query-halves)
    software pipeline; every kc slot issues one score matmul pair
    (row-tiled, concurrent), one AV matmul pair (col-tiled K=128 into
    a single accumulator bank = final concat layout) and ~2
    projection-fill steps, pacing the scalar engine's exp (~1.1us per
    [128,1024] tile)
  - each unit's tail (last two AV groups, denominator reduce via
    K=128 ones-matmuls over a DVE/GpSimd-split esum, reciprocal
    riding the PSUM evacuation, DRAM-bounce broadcast, normalize
    multiplies straight out of the AV bank) is deferred into fixed
    slots of the NEXT unit so no engine stalls at a unit boundary
  - ~16 junk matmuls at t=0 release the HAM clock throttle before the
    real prologue; xT spreads across three DMA queues; the pair-0
    q/k projections are the only serial prologue -- all v chunks and
    pair-1's projections ride as fill inside unit 0
  - b_v folds into b_proj host-side (sum of attention weights is 1),
    so v evacuation is a pure copy on the otherwise-idle scalar
    queue; qk bias evacuation stays on DVE
  - output projection of the first token half fills the qc=1 units;
    the trailing four chunks run after the final flush, whose
    broadcast uses a K=1 ones-matmul instead of the DRAM bounce
"""

import sys

if "/opt/trn_rl_repo" not in sys.path:
    sys.path.insert(0, "/opt/trn_rl_repo")

from collections import deque
from contextlib import ExitStack

import ml_dtypes
import numpy as np

import concourse.bass as bass
import concourse.tile as tile
from concourse import bacc, mybir
from concourse import bass_utils

N_HEAD = 12
B = 8
T = 1024
C = 768
HD = 64
KO = C // 128          # 6 contraction chunks of 128
TC = T // 128          # 8 token chunks of 128
QN = T // 512          # 2 query chunks of 512
NPAIR = N_HEAD // 2    # 6 head pairs

F32 = mybir.dt.float32
F32R = mybir.dt.float32r
BF16 = mybir.dt.bfloat16
AF = mybir.ActivationFunctionType
ADD = mybir.AluOpType.add

_cache: dict = {}
BF = ml_dtypes.bfloat16


def _emit_kernel(tc_ctx, aps):
    nc = tc_ctx.nc
    ctx = aps["ctx"]
    xT_d, wv_d, wqk_d, wp_d, cF_d, cB_d, y_d = (
        aps["xT"], aps["wv"], aps["wqk"], aps["wp"], aps["cF"], aps["cB"],
        aps["y"],
    )

    const = ctx.enter_context(tc_ctx.tile_pool(name="const", bufs=1))
    e_pool = ctx.enter_context(tc_ctx.tile_pool(name="e", bufs=5))
    es_pool = ctx.enter_context(tc_ctx.tile_pool(name="es", bufs=2))
    dr_pool = ctx.enter_context(tc_ctx.tile_pool(name="dr", bufs=2))
    rbw_pool = ctx.enter_context(tc_ctx.tile_pool(name="rbw", bufs=2))
    rd_pool = ctx.enter_context(tc_ctx.tile_pool(name="rd", bufs=2, space="DRAM"))
    out_pool = ctx.enter_context(tc_ctx.tile_pool(name="out", bufs=2))

    # PSUM: 8 banks = scores 2x[128,1024] (4; the ring also lends
    # slots to the per-unit denominator tiles) + AV accumulators (2) +
    # qkv/proj fill accumulators (2)
    sc_ps = ctx.enter_context(tc_ctx.tile_pool(name="scps", bufs=2, space="PSUM"))
    av_ps = ctx.enter_context(tc_ctx.tile_pool(name="avps", bufs=2, space="PSUM"))
    acc_ps = ctx.enter_context(tc_ctx.tile_pool(name="accps", bufs=2, space="PSUM"))

    # ---- persistent SBUF tensors -------------------------------------
    xT_sb = const.tile([128, KO, T], BF16)
    wv_sb = const.tile([128, KO, C], BF16)
    wqk_sb = const.tile([128, 12, KO, 128], BF16)
    wp_sb = const.tile([128, KO, C], BF16)
    qk_sb = const.tile([128, KO, 2, T], BF16)   # [pair, half(q/k), t]
    v_sb = const.tile([128, TC, N_HEAD, HD], BF16)
    # cF: [:,0:12]=bqk | [:,12:20]=mb | [0,20:148]=ones (f32r row for
    # the final flush's K=1 broadcast matmul)
    cF_sb = const.tile([128, 148], F32R)
    # cB: [:,0:128]=ones | [:,128:896]=b_v | [:,896:1664]=b_proj
    # (biases replicated across partitions so DVE adds read lane-local)
    cB_sb = const.tile([128, 1664], BF16)

    # ---- PE pre-warm --------------------------------------------------
    # ~16 junk matmuls release the HAM clock throttle (K=4/8 -> 8/8,
    # ~3.4us of sustained activity) and keep the PE busy through the
    # input-DMA window, so the real prologue matmuls run at 2.4 GHz
    junkw = const.tile([128, 640], BF16)
    junkp = sc_ps.tile([128, 512], F32, tag="sc", name="junkp")
    nc.gpsimd.memset(junkw[:], 0)
    for _ in range(16):
        nc.tensor.matmul(
            junkp[:, 0:256], junkw[:, 0:128], junkw[:, 128:384],
            start=True, stop=True,
        )

    # ---- input DMAs, spread across queues ----------------------------
    # priority: xT + pair-0 wqk feed the first matmuls; wv feeds the
    # v chunks from ~7us; the rest of wqk is needed from unit 0's fill
    nc.gpsimd.dma_start(cF_sb[:], cF_d)
    nc.gpsimd.dma_start(cB_sb[:], cB_d)
    xT_r = xT_d.rearrange("p (ko t) -> p ko t", ko=KO)
    wv_r = wv_d.rearrange("p (ko n) -> p ko n", ko=KO)
    wqk_r = wqk_d.rearrange("p (m ko n) -> p m ko n", m=12, ko=KO)
    # xT spreads over three queues so the prologue isn't serialized
    # behind one queue's ~1.2us-per-chunk service rate
    nc.scalar.dma_start(wqk_sb[:, 0], wqk_r[:, 0])
    nc.scalar.dma_start(wqk_sb[:, 6], wqk_r[:, 6])
    xT_q = [nc.sync, nc.scalar, nc.gpsimd]
    for ko in range(KO):
        xT_q[ko % 3].dma_start(xT_sb[:, ko], xT_r[:, ko])
    for ko in range(KO):
        nc.sync.dma_start(wv_sb[:, ko], wv_r[:, ko])
    for j in range(1, NPAIR):
        nc.gpsimd.dma_start(wqk_sb[:, j], wqk_r[:, j])
        nc.gpsimd.dma_start(wqk_sb[:, 6 + j], wqk_r[:, 6 + j])

    segs = [(0, 512), (512, 256)]
    bqk_sb = cF_sb[:, 0:12]
    mb_sb = cF_sb[:, 12:20]

    def psum_pair(name, ring):
        # a (512, 256)-wide accumulator pair from either psum ring
        if ring == 0:
            return [
                acc_ps.tile([128, 512], F32, tag="acc", name=f"{name}_{i}")
                for i in range(2)
            ]
        big = sc_ps.tile([128, 1024], F32, tag="sc", name=name)
        return [big[:, 0:512], big[:, 512:1024]]

    # ---- phase 1b: v[t, c'] for c' in [1536, 2304) -------------------
    def v_chunk_steps(tcc, ring=0):
        pss = psum_pair(f"ps1b_{tcc}", ring)

        def ko_step(ko):
            def f():
                for i, (off, w) in enumerate(segs):
                    nc.tensor.matmul(
                        pss[i][:, :w],
                        xT_sb[:, ko, tcc * 128 : (tcc + 1) * 128],
                        wv_sb[:, ko, off : off + w],
                        start=(ko == 0),
                        stop=(ko == KO - 1),
                    )
            return f

        def out_step():
            # pure copy: out_h = sum_k a_k (v_k + b_v) = (sum a_k v_k)
            # + b_v since sum a = 1, so b_v folds into b_proj host-side.
            # the two segments split across DVE and Scalar: unit 0 runs
            # all eight v chunks, and either queue alone would be
            # oversubscribed there (scalar also carries the exps)
            for i, (off, w) in enumerate(segs):
                eng = nc.vector.tensor_copy if i == 0 else nc.scalar.copy
                eng(
                    out=v_sb[:, tcc, off // HD : (off + w) // HD, :],
                    in_=pss[i][:, :w].rearrange("p (h d) -> p h d", d=HD),
                )

        return [ko_step(ko) for ko in range(KO)] + [out_step]

    # ---- phase 1a: qkT chunk m (fill-steppable) ----------------------
    def qk_chunk_steps(j, half, m, ring=0):
        # half 0 -> qT chunk (m = j), half 1 -> kT chunk (m = 6 + j)
        pss = psum_pair(f"ps1a_{m}", ring)

        def ko_step(ko):
            def f():
                for nq in range(QN):
                    nc.tensor.matmul(
                        pss[nq],
                        wqk_sb[:, m, ko, :],
                        xT_sb[:, ko, nq * 512 : (nq + 1) * 512],
                        start=(ko == 0),
                        stop=(ko == KO - 1),
                    )
            return f

        def bias_step():
            # stays on DVE: routing this through the scalar queue
            # would park the next unit's score inputs behind a full
            # unit of exp tiles in the scalar FIFO
            for nq in range(QN):
                nc.vector.tensor_tensor(
                    qk_sb[:, j, half, nq * 512 : (nq + 1) * 512],
                    pss[nq],
                    bqk_sb[:, m : m + 1].to_broadcast((128, 512)),
                    ADD,
                )

        return [ko_step(ko) for ko in range(KO)] + [bias_step]

    # ---- phase 4: one token chunk of y = concatT.T @ W_proj ----------
    def proj_chunk_steps(tcc, ring=0, yq=None):
        pss = psum_pair(f"ps4_{tcc}", ring)
        o_sb = out_pool.tile([128, C], BF16, tag="out", name=f"o_{tcc}")

        def ko_step(ko):
            def f():
                for i, (off, w) in enumerate(segs):
                    nc.tensor.matmul(
                        pss[i][:, :w],
                        qk_sb[:, ko, 0, tcc * 128 : (tcc + 1) * 128],
                        wp_sb[:, ko, off : off + w],
                        start=(ko == 0),
                        stop=(ko == KO - 1),
                    )
            return f

        def out_step():
            for i, (off, w) in enumerate(segs):
                nc.vector.tensor_tensor(
                    o_sb[:, off : off + w],
                    pss[i][:, :w],
                    cB_sb[:, 896 + off : 896 + off + w],
                    ADD,
                )
            (yq or nc.sync).dma_start(y_d[tcc * 128 : (tcc + 1) * 128, :], o_sb[:])

        return [ko_step(ko) for ko in range(KO)] + [out_step]

    # ---- attention unit: one (head pair, query half) -----------------
    class AttnUnit:
        def __init__(self, j, qc):
            self.j, self.qc = j, qc
            self.qsl = slice(qc * 512, (qc + 1) * 512)
            self.avp = av_ps.tile(
                [128, 512], F32, tag="av", name=f"avp_{j}_{qc}"
            )
            # esum splits into a 6-term DVE chain and one independent
            # GpSimd partial (e4+e5) to shave the near-saturated DVE;
            # the denominator matmuls accumulate both partials
            self.esum = es_pool.tile(
                [128, 1024], BF16, tag="es", name=f"es_{j}_{qc}"
            )
            self.esg = es_pool.tile(
                [128, 1024], BF16, tag="esg", name=f"esg_{j}_{qc}"
            )
            self.e = {}
            self.drow = None
            self.rbw = None

        def sc_act(self, kc):
            j, qc = self.j, self.qc
            ksl = slice(kc * 128, (kc + 1) * 128)
            sc = sc_ps.tile(
                [128, 1024], F32, tag="sc", name=f"sc_{j}_{qc}_{kc}"
            )
            nc.tensor.matmul(
                sc[:, 0:512], qk_sb[0:64, j, 1, ksl], qk_sb[0:64, j, 0, self.qsl],
                start=True, stop=True, tile_position=(0, 0),
            )
            nc.tensor.matmul(
                sc[:, 512:1024], qk_sb[64:128, j, 1, ksl],
                qk_sb[64:128, j, 0, self.qsl],
                start=True, stop=True, tile_position=(64, 0),
            )
            e = e_pool.tile([128, 1024], BF16, tag="e", name=f"e_{j}_{qc}_{kc}")
            nc.scalar.activation(
                e, sc, AF.Exp, bias=mb_sb[:, kc : kc + 1], scale=0.125
            )
            # bf16 accumulation; the per-element rounding error washes
            # out in the 128-row partition reduction that follows
            if kc == 1:
                nc.vector.tensor_tensor(self.esum[:], self.e[0][:], e[:], ADD)
            elif kc in (2, 3, 6, 7):
                nc.vector.tensor_tensor(self.esum[:], self.esum[:], e[:], ADD)
            elif kc == 5:
                nc.gpsimd.tensor_tensor(self.esg[:], self.e[4][:], e[:], ADD)
            self.e[kc] = e

        def av(self, kc):
            # two K=128 col-tiled matmuls (one per head) run
            # concurrently; head b lands on partitions 64:128 of the
            # single accumulator bank (= final concat layout).
            # start=True clears the whole bank, so h=1's kc=0 matmul
            # lands on cleared has_written bits -> fresh overwrite.
            j = self.j
            e = self.e.pop(kc)
            for h in range(2):
                nc.tensor.matmul(
                    self.avp[h * 64 : (h + 1) * 64, :],
                    v_sb[:, kc, 2 * j + h, :],
                    e[:, h * 512 : (h + 1) * 512],
                    start=(kc == 0),
                    stop=(kc == TC - 1),
                    tile_position=(0, h * 64),
                    skip_group_check=True,
                )

        def dps_drow(self):
            # denominators: partition-reduce esum via K=128 ones
            # matmuls; the reciprocal rides the PSUM evacuation (drow
            # holds 1/denominator).  dps tiles borrow sc-ring slots;
            # they are emitted BEFORE the slot's own score matmul so
            # they land on banks whose ACTIVATEs already retired
            # (otherwise the PE queue serializes behind the scalar
            # engine for a full exp tile).
            j, qc = self.j, self.qc
            self.drow = dr_pool.tile(
                [1, 1024], F32, tag="dr", name=f"dr_{j}_{qc}"
            )
            for h in range(2):
                dps = sc_ps.tile(
                    [1, 512], F32, tag="sc", name=f"dps_{j}_{qc}_{h}"
                )
                hsl = slice(h * 512, (h + 1) * 512)
                nc.tensor.matmul(
                    dps, cB_sb[:, 0:1], self.esum[:, hsl],
                    start=True, stop=False,
                )
                nc.tensor.matmul(
                    dps, cB_sb[:, 0:1], self.esg[:, hsl],
                    start=False, stop=True,
                )
                nc.vector.reciprocal_approx_fast(
                    out=self.drow[0:1, hsl], in_=dps[:]
                )

        def norm_dma(self):
            # broadcast 1/denominator over all 128 partitions via a
            # DRAM bounce (gpsimd queue; lands well before norm_muls)
            j, qc = self.j, self.qc
            rd = rd_pool.tile([1, 1024], F32, tag="rd", name=f"rd_{j}_{qc}")
            nc.gpsimd.dma_start(rd[:], self.drow[:])
            self.rbw = rbw_pool.tile(
                [128, 1024], F32, tag="rbw", name=f"rbw_{j}_{qc}"
            )
            nc.gpsimd.dma_start(self.rbw[:], rd.to_broadcast((128, 1024)))

        def norm_bcast_pe(self):
            # flush path: broadcast 1/denominator via K=1 ones-matmul
            # (no DRAM-bounce latency at the end of the program); the
            # DVE copy stages it in SBUF since the normalize multiply
            # may read only one PSUM operand
            j, qc = self.j, self.qc
            rbsb = rbw_pool.tile(
                [128, 1024], F32, tag="rbsb", name=f"rbsb_{j}_{qc}"
            )
            drow_r = dr_pool.tile(
                [1, 1024], F32R, tag="drr", name=f"drr_{j}_{qc}"
            )
            nc.vector.tensor_copy(out=drow_r[:], in_=self.drow[:])
            for h in range(2):
                rbp = sc_ps.tile(
                    [128, 512], F32, tag="sc", name=f"rbp_{j}_{qc}_{h}"
                )
                nc.tensor.matmul(
                    rbp, cF_sb[0:1, 20:148],
                    drow_r[0:1, h * 512 : (h + 1) * 512],
                    start=True, stop=True,
                )
                nc.vector.tensor_copy(
                    out=rbsb[:, h * 512 : (h + 1) * 512], in_=rbp[:]
                )
            self.rbsb = rbsb

        def norm_muls(self, pe=False):
            # normalize straight out of the AV accumulator bank (one
            # PSUM operand per DVE op: avp is the PSUM side)
            j = self.j
            if pe:
                rb0 = self.rbsb[0:64, 0:512]
                rb1 = self.rbsb[64:128, 512:1024]
            else:
                rb0 = self.rbw[0:64, 0:512]
                rb1 = self.rbw[64:128, 512:1024]
            nc.vector.tensor_mul(
                out=qk_sb[0:64, j, 0, self.qsl],
                in0=self.avp[0:64, :],
                in1=rb0,
            )
            nc.vector.tensor_mul(
                out=qk_sb[64:128, j, 0, self.qsl],
                in0=self.avp[64:128, :],
                in1=rb1,
            )

    # ---- schedule ----------------------------------------------------
    # prologue: just the pair-0 q/k projections -- everything else
    # rides as unit-0 fill so the first exp fires as early as possible
    for step in qk_chunk_steps(0, 0, 0, ring=0):
        step()
    for step in qk_chunk_steps(0, 1, 6, ring=1):
        step()

    # unit 0's fill: v chunks 2-7 then pair-1 projections.  qc=0 units
    # j>=1 fill with pair j+1's projections; qc=1 units fill with the
    # first-half output projection.
    # pair-1's projections are interleaved between v chunks so their
    # bias evacuations retire well before the unit-0/1 boundary
    unit0_fill = deque()
    unit0_fill.extend(v_chunk_steps(0, ring=0))
    unit0_fill.extend(v_chunk_steps(1, ring=0))
    unit0_fill.extend(v_chunk_steps(2, ring=0))
    # pair-1's projections go mid-fill so unit 1's scores aren't gated
    # on the tail of unit 0's fill; later v chunks still beat their
    # AV deadlines (v[c] needed by slot c+2)
    unit0_fill.extend(qk_chunk_steps(1, 0, 1))
    unit0_fill.extend(qk_chunk_steps(1, 1, 7))
    unit0_fill.extend(v_chunk_steps(3, ring=0))
    unit0_fill.extend(v_chunk_steps(4, ring=0))
    unit0_fill.extend(v_chunk_steps(5, ring=0))
    unit0_fill.extend(v_chunk_steps(6, ring=0))
    unit0_fill.extend(v_chunk_steps(7, ring=0))

    prev = None
    gfill = None
    gslots = [NPAIR * TC]
    for qc in range(QN):
        for j in range(NPAIR):
            u = AttnUnit(j, qc)
            cross = qc == 1 and j == 0
            if cross:
                gfill = deque()
                for tcc in range(TC // 2):
                    gfill.extend(proj_chunk_steps(tcc, ring=0, yq=nc.gpsimd))
                gslots = [NPAIR * TC]
            if qc == 0:
                if j == 0:
                    fill, cap = unit0_fill, 12
                elif j == 1:
                    # wp arrives mid-flight: needed from qc=1, and
                    # loading it at t=0 would steal HBM bandwidth
                    nc.sync.dma_start(
                        wp_sb[:], wp_d.rearrange("p (ko n) -> p ko n", ko=KO)
                    )
                    fill = deque(
                        qk_chunk_steps(j + 1, 0, j + 1)
                        + qk_chunk_steps(j + 1, 1, 7 + j)
                    )
                    cap = 3
                elif j < NPAIR - 1:
                    fill = deque(
                        qk_chunk_steps(j + 1, 0, j + 1)
                        + qk_chunk_steps(j + 1, 1, 7 + j)
                    )
                    cap = 3
                else:
                    fill, cap = deque(), 3
            else:
                fill, cap = gfill, 2
            # fixed slots for the previous unit's tail; the qc=0->1
            # crossing runs them earlier so the qc=1 projection fill
            # (which reads pair 5's normalized output) isn't blocked
            sched = (
                {0: "av6", 1: "av7", 2: "dps", 3: "dma", 5: "muls"}
                if cross
                else {0: "av6", 1: "av7", 2: "dps", 4: "dma", 7: "muls"}
            )
            for kc in range(TC):
                # previous unit's PE-side tail events go ahead of this
                # slot's score matmul: av6/av7 are immediately
                # runnable, and dps must grab sc-ring banks whose
                # ACTIVATE retired.  The DVE-side events (muls) go
                # AFTER esum so they don't delay the esum chain in the
                # strict-FIFO DVE queue.
                ev = sched.get(kc) if prev is not None else None
                if ev == "av6":
                    prev.av(6)
                elif ev == "av7":
                    prev.av(7)
                elif ev == "dps":
                    prev.dps_drow()
                u.sc_act(kc)
                if ev == "dma":
                    prev.norm_dma()
                elif ev == "muls":
                    prev.norm_muls()
                if kc >= 2:
                    u.av(kc - 2)
                if fill:
                    if qc == 0:
                        denom = TC - kc
                        n = min(cap, max(1, -(-len(fill) // denom)))
                    else:
                        # ~1 step per slot while the backlog is deep,
                        # every other slot once it thins, leaving a few
                        # for the final-flush drain so the PE never
                        # idles long enough to cool
                        n = (
                            1
                            if (len(fill) > 14 or gslots[0] % 2 == 0)
                            else 0
                        )
                    for _ in range(n):
                        if fill:
                            fill.popleft()()
                gslots[0] -= 1
            if qc == 0:
                while fill:
                    fill.popleft()()
            prev = u

    # ---- final unit's tail + second-half output projection -----------
    # interleave the 4 trailing proj chunks ko-wise (2 on the acc ring,
    # 2 on the freed sc ring) so the last normalization's DRAM bounce
    # hides under matmul work; the ko=5 terms (which read pair 5's
    # normalized output) come after norm_muls.
    prev.av(6)
    prev.av(7)
    prev.dps_drow()
    # reserved gfill steps drain here: they execute while the flush's
    # reciprocal / f32r rounding retire on the DVE, keeping the PE warm
    while gfill:
        gfill.popleft()()
    prev.norm_bcast_pe()
    prev.norm_muls(pe=True)
    tail_q = [nc.sync, nc.gpsimd, nc.scalar, nc.sync]
    tails = [
        proj_chunk_steps(tcc, ring=(0 if tcc < 6 else 1), yq=tail_q[tcc - 4])
        for tcc in range(TC // 2, TC)
    ]
    for ko in range(KO):
        for t in tails:
            t[ko]()
    for t in tails:
        t[KO]()


def _get_program():
    if "nc" in _cache:
        return _cache["nc"]
    nc = bacc.Bacc(
        "TRN2", target_bir_lowering=False, debug=False, enable_asserts=True
    )
    aps = {
        "xT": nc.dram_tensor("xT", [128, KO * T], BF16, kind="ExternalInput").ap(),
        "wv": nc.dram_tensor("wv", [128, KO * C], BF16, kind="ExternalInput").ap(),
        "wqk": nc.dram_tensor(
            "wqk", [128, 12 * KO * 128], BF16, kind="ExternalInput"
        ).ap(),
        "wp": nc.dram_tensor("wp", [128, KO * C], BF16, kind="ExternalInput").ap(),
        "cF": nc.dram_tensor("cF", [128, 148], F32R, kind="ExternalInput").ap(),
        "cB": nc.dram_tensor("cB", [128, 1664], BF16, kind="ExternalInput").ap(),
        "y": nc.dram_tensor("y", [T, C], BF16, kind="ExternalOutput").ap(),
    }
    with tile.TileContext(nc) as tc_ctx, ExitStack() as ctx:
        aps["ctx"] = ctx
        _emit_kernel(tc_ctx, aps)
    nc.compile()
    _cache["nc"] = nc
    return nc


def _p_major(a, ko=KO):
    # [(ko p), n] -> [p, (ko n)] partition-major layout
    n = a.shape[1]
    return np.ascontiguousarray(
        a.reshape(ko, 128, n).transpose(1, 0, 2).reshape(128, ko * n)
    )


def _make_in_maps(inputs):
    x = np.asarray(inputs["x"], np.float32)
    mask = np.asarray(inputs["attn_mask"])
    Wa = np.asarray(inputs["W_attn"], np.float32)
    ba = np.asarray(inputs["b_attn"], np.float32)
    Wp = np.asarray(inputs["W_proj"], np.float32)
    bp = np.asarray(inputs["b_proj"], np.float32)

    wv = _p_major(Wa[:, 2 * C :]).astype(BF)
    wqk = np.concatenate(
        [_p_major(Wa[:, m * 128 : (m + 1) * 128]) for m in range(12)], axis=1
    ).astype(BF)
    wp = _p_major(Wp).astype(BF)

    cB = np.zeros((128, 1664), BF)
    cB[:, 0:128] = 1
    # b_v folds into the projection bias (sum of attention weights is
    # 1, so out_h picks up exactly one b_v): bp' = bp + b_v @ W_proj
    bp_eff = bp + ba[2 * C :] @ Wp
    cB[:, 896:1664] = bp_eff.astype(BF)[None, :]

    cF0 = np.zeros((128, 148), np.float32)
    cF0[:, 0:12] = ba[: 2 * C].reshape(12, 128).T
    cF0[0, 20:148] = 1.0
    in_maps = []
    for b in range(B):
        cF = cF0.copy()
        mb = np.where(mask[b] == 0, np.float32(-30.0), np.float32(0.0))
        cF[:, 12:20] = mb.reshape(TC, 128).T
        in_maps.append(
            {
                "xT": _p_major(np.ascontiguousarray(x[b].T)).astype(BF),
                "wv": wv,
                "wqk": wqk,
                "wp": wp,
                "cF": cF,
                "cB": cB,
            }
        )
    return in_maps


def _run(inputs, trace=False):
    nc = _get_program()
    in_maps = _make_in_maps(inputs)
    res = bass_utils.run_bass_kernel_spmd(
        nc, in_maps, core_ids=list(range(B)), trace=trace
    )
    y = np.stack(
        [res.results[b]["y"].astype(np.float32) for b in range(B)], axis=0
    )
    return y, res


def kernel(**inputs) -> np.ndarray:
    y, _ = _run(inputs, trace=False)
    return y


# revision 53
# speedup vs baseline: 1.0310x; 1.0310x over previous
"""Bass/Trainium2 kernel for a 12-head self-attention block
(B=8, T=1024, C=768), data-parallel across 8 NeuronCores (one batch
element per core).

Per-core computation (batch element b):
  qkv   = x @ W_attn + b_attn            [T, 3C]
  scoresT[k, q] = k_h . q_h / 8 (+ mask bias), keys on partitions
  e     = exp(scoresT)                   (unnormalized)
  out_h = (v_h.T @ e_h) / (sum_k e_h)
  y     = concat(out_h) @ W_proj + b_proj

v6 design (all matmul operands bf16, fp32 PSUM accumulation):
  - attention is a uniform 12-unit (6 head-pairs x 2 query-halves)
    software pipeline; every kc slot issues one score matmul pair
    (row-tiled, concurrent), one AV matmul pair (col-tiled K=128 into
    a single accumulator bank = final concat layout) and ~2
    projection-fill steps, pacing the scalar engine's exp (~1.1us per
    [128,1024] tile)
  - each unit's tail (last two AV groups, denominator reduce via
    K=128 ones-matmuls over a DVE/GpSimd-split esum, reciprocal
    riding the PSUM evacuation, DRAM-bounce broadcast, normalize
    multiplies straight out of the AV bank) is deferred into fixed
    slots of the NEXT unit so no engine stalls at a unit boundary
  - ~16 junk matmuls at t=0 release the HAM clock throttle before the
    real prologue; xT spreads across three DMA queues; the pair-0
    q/k projections are the only serial prologue -- all v chunks and
    pair-1's projections ride as fill inside unit 0
  - b_v folds into b_proj host-side (sum of attention weights is 1),
    so v evacuation is a pure copy on the otherwise-idle scalar
    queue; qk bias evacuation stays on DVE
  - output projection of the first token half fills the qc=1 units;
    the trailing four chunks run after the final flush, whose
    broadcast uses a K=1 ones-matmul instead of the DRAM bounce
"""

import sys

if "/opt/trn_rl_repo" not in sys.path:
    sys.path.insert(0, "/opt/trn_rl_repo")

from collections import deque
from contextlib import ExitStack

import ml_dtypes
import numpy as np

import concourse.bass as bass
import concourse.tile as tile
from concourse import bacc, mybir
from concourse import bass_utils

N_HEAD = 12
B = 8
T = 1024
C = 768
HD = 64
KO = C // 128          # 6 contraction chunks of 128
TC = T // 128          # 8 token chunks of 128
QN = T // 512          # 2 query chunks of 512
NPAIR = N_HEAD // 2    # 6 head pairs

F32 = mybir.dt.float32
F32R = mybir.dt.float32r
BF16 = mybir.dt.bfloat16
AF = mybir.ActivationFunctionType
ADD = mybir.AluOpType.add

_cache: dict = {}
BF = ml_dtypes.bfloat16


def _emit_kernel(tc_ctx, aps):
    nc = tc_ctx.nc
    ctx = aps["ctx"]
    xT_d, wv_d, wqk_d, wp_d, cF_d, cB_d, y_d = (
        aps["xT"], aps["wv"], aps["wqk"], aps["wp"], aps["cF"], aps["cB"],
        aps["y"],
    )

    const = ctx.enter_context(tc_ctx.tile_pool(name="const", bufs=1))
    e_pool = ctx.enter_context(tc_ctx.tile_pool(name="e", bufs=5))
    es_pool = ctx.enter_context(tc_ctx.tile_pool(name="es", bufs=2))
    dr_pool = ctx.enter_context(tc_ctx.tile_pool(name="dr", bufs=2))
    rbw_pool = ctx.enter_context(tc_ctx.tile_pool(name="rbw", bufs=2))
    rd_pool = ctx.enter_context(tc_ctx.tile_pool(name="rd", bufs=2, space="DRAM"))
    out_pool = ctx.enter_context(tc_ctx.tile_pool(name="out", bufs=2))

    # PSUM: 8 banks = scores 2x[128,1024] (4; the ring also lends
    # slots to the per-unit denominator tiles) + AV accumulators (2) +
    # qkv/proj fill accumulators (2)
    sc_ps = ctx.enter_context(tc_ctx.tile_pool(name="scps", bufs=2, space="PSUM"))
    av_ps = ctx.enter_context(tc_ctx.tile_pool(name="avps", bufs=2, space="PSUM"))
    acc_ps = ctx.enter_context(tc_ctx.tile_pool(name="accps", bufs=2, space="PSUM"))

    # ---- persistent SBUF tensors -------------------------------------
    xT_sb = const.tile([128, KO, T], BF16)
    wv_sb = const.tile([128, KO, C], BF16)
    wqk_sb = const.tile([128, 12, KO, 128], BF16)
    wp_sb = const.tile([128, KO, C], BF16)
    qk_sb = const.tile([128, KO, 2, T], BF16)   # [pair, half(q/k), t]
    v_sb = const.tile([128, TC, N_HEAD, HD], BF16)
    # cF: [:,0:12]=bqk | [:,12:20]=mb | [0,20:148]=ones (f32r row for
    # the final flush's K=1 broadcast matmul)
    cF_sb = const.tile([128, 148], F32R)
    # cB: [:,0:128]=ones | [:,128:896]=b_v | [:,896:1664]=b_proj
    # (biases replicated across partitions so DVE adds read lane-local)
    cB_sb = const.tile([128, 1664], BF16)

    # ---- PE pre-warm --------------------------------------------------
    # ~16 junk matmuls release the HAM clock throttle (K=4/8 -> 8/8,
    # ~3.4us of sustained activity) and keep the PE busy through the
    # input-DMA window, so the real prologue matmuls run at 2.4 GHz
    junkw = const.tile([128, 640], BF16)
    junkp = sc_ps.tile([128, 512], F32, tag="sc", name="junkp")
    nc.gpsimd.memset(junkw[:], 0)
    for _ in range(16):
        nc.tensor.matmul(
            junkp[:, 0:256], junkw[:, 0:128], junkw[:, 128:384],
            start=True, stop=True,
        )

    # ---- input DMAs, spread across queues ----------------------------
    # priority: xT + pair-0 wqk feed the first matmuls; wv feeds the
    # v chunks from ~7us; the rest of wqk is needed from unit 0's fill
    nc.gpsimd.dma_start(cF_sb[:], cF_d)
    nc.gpsimd.dma_start(cB_sb[:], cB_d)
    xT_r = xT_d.rearrange("p (ko t) -> p ko t", ko=KO)
    wv_r = wv_d.rearrange("p (ko n) -> p ko n", ko=KO)
    wqk_r = wqk_d.rearrange("p (m ko n) -> p m ko n", m=12, ko=KO)
    # xT spreads over three queues so the prologue isn't serialized
    # behind one queue's ~1.2us-per-chunk service rate
    nc.scalar.dma_start(wqk_sb[:, 0], wqk_r[:, 0])
    nc.scalar.dma_start(wqk_sb[:, 6], wqk_r[:, 6])
    xT_q = [nc.sync, nc.scalar, nc.gpsimd]
    for ko in range(KO):
        xT_q[ko % 3].dma_start(xT_sb[:, ko], xT_r[:, ko])
    for ko in range(KO):
        nc.sync.dma_start(wv_sb[:, ko], wv_r[:, ko])
    for j in range(1, NPAIR):
        nc.gpsimd.dma_start(wqk_sb[:, j], wqk_r[:, j])
        nc.gpsimd.dma_start(wqk_sb[:, 6 + j], wqk_r[:, 6 + j])

    segs = [(0, 512), (512, 256)]
    bqk_sb = cF_sb[:, 0:12]
    mb_sb = cF_sb[:, 12:20]

    def psum_pair(name, ring):
        # a (512, 256)-wide accumulator pair from either psum ring
        if ring == 0:
            return [
                acc_ps.tile([128, 512], F32, tag="acc", name=f"{name}_{i}")
                for i in range(2)
            ]
        big = sc_ps.tile([128, 1024], F32, tag="sc", name=name)
        return [big[:, 0:512], big[:, 512:1024]]

    # ---- phase 1b: v[t, c'] for c' in [1536, 2304) -------------------
    def v_chunk_steps(tcc, ring=0):
        pss = psum_pair(f"ps1b_{tcc}", ring)

        def ko_step(ko):
            def f():
                for i, (off, w) in enumerate(segs):
                    nc.tensor.matmul(
                        pss[i][:, :w],
                        xT_sb[:, ko, tcc * 128 : (tcc + 1) * 128],
                        wv_sb[:, ko, off : off + w],
                        start=(ko == 0),
                        stop=(ko == KO - 1),
                    )
            return f

        def out_step():
            # pure copy: out_h = sum_k a_k (v_k + b_v) = (sum a_k v_k)
            # + b_v since sum a = 1, so b_v folds into b_proj host-side
            for i, (off, w) in enumerate(segs):
                nc.scalar.copy(
                    out=v_sb[:, tcc, off // HD : (off + w) // HD, :],
                    in_=pss[i][:, :w].rearrange("p (h d) -> p h d", d=HD),
                )

        return [ko_step(ko) for ko in range(KO)] + [out_step]

    # ---- phase 1a: qkT chunk m (fill-steppable) ----------------------
    def qk_chunk_steps(j, half, m, ring=0):
        # half 0 -> qT chunk (m = j), half 1 -> kT chunk (m = 6 + j)
        pss = psum_pair(f"ps1a_{m}", ring)

        def ko_step(ko):
            def f():
                for nq in range(QN):
                    nc.tensor.matmul(
                        pss[nq],
                        wqk_sb[:, m, ko, :],
                        xT_sb[:, ko, nq * 512 : (nq + 1) * 512],
                        start=(ko == 0),
                        stop=(ko == KO - 1),
                    )
            return f

        def bias_step():
            # stays on DVE: routing this through the scalar queue
            # would park the next unit's score inputs behind a full
            # unit of exp tiles in the scalar FIFO
            for nq in range(QN):
                nc.vector.tensor_tensor(
                    qk_sb[:, j, half, nq * 512 : (nq + 1) * 512],
                    pss[nq],
                    bqk_sb[:, m : m + 1].to_broadcast((128, 512)),
                    ADD,
                )

        return [ko_step(ko) for ko in range(KO)] + [bias_step]

    # ---- phase 4: one token chunk of y = concatT.T @ W_proj ----------
    def proj_chunk_steps(tcc, ring=0, yq=None):
        pss = psum_pair(f"ps4_{tcc}", ring)
        o_sb = out_pool.tile([128, C], BF16, tag="out", name=f"o_{tcc}")

        def ko_step(ko):
            def f():
                for i, (off, w) in enumerate(segs):
                    nc.tensor.matmul(
                        pss[i][:, :w],
                        qk_sb[:, ko, 0, tcc * 128 : (tcc + 1) * 128],
                        wp_sb[:, ko, off : off + w],
                        start=(ko == 0),
                        stop=(ko == KO - 1),
                    )
            return f

        def out_step():
            for i, (off, w) in enumerate(segs):
                nc.vector.tensor_tensor(
                    o_sb[:, off : off + w],
                    pss[i][:, :w],
                    cB_sb[:, 896 + off : 896 + off + w],
                    ADD,
                )
            (yq or nc.sync).dma_start(y_d[tcc * 128 : (tcc + 1) * 128, :], o_sb[:])

        return [ko_step(ko) for ko in range(KO)] + [out_step]

    # ---- attention unit: one (head pair, query half) -----------------
    class AttnUnit:
        def __init__(self, j, qc):
            self.j, self.qc = j, qc
            self.qsl = slice(qc * 512, (qc + 1) * 512)
            self.avp = av_ps.tile(
                [128, 512], F32, tag="av", name=f"avp_{j}_{qc}"
            )
            # esum splits into a 6-term DVE chain and one independent
            # GpSimd partial (e4+e5) to shave the near-saturated DVE;
            # the denominator matmuls accumulate both partials
            self.esum = es_pool.tile(
                [128, 1024], BF16, tag="es", name=f"es_{j}_{qc}"
            )
            self.esg = es_pool.tile(
                [128, 1024], BF16, tag="esg", name=f"esg_{j}_{qc}"
            )
            self.e = {}
            self.drow = None
            self.rbw = None

        def sc_act(self, kc):
            j, qc = self.j, self.qc
            ksl = slice(kc * 128, (kc + 1) * 128)
            sc = sc_ps.tile(
                [128, 1024], F32, tag="sc", name=f"sc_{j}_{qc}_{kc}"
            )
            nc.tensor.matmul(
                sc[:, 0:512], qk_sb[0:64, j, 1, ksl], qk_sb[0:64, j, 0, self.qsl],
                start=True, stop=True, tile_position=(0, 0),
            )
            nc.tensor.matmul(
                sc[:, 512:1024], qk_sb[64:128, j, 1, ksl],
                qk_sb[64:128, j, 0, self.qsl],
                start=True, stop=True, tile_position=(64, 0),
            )
            e = e_pool.tile([128, 1024], BF16, tag="e", name=f"e_{j}_{qc}_{kc}")
            nc.scalar.activation(
                e, sc, AF.Exp, bias=mb_sb[:, kc : kc + 1], scale=0.125
            )
            # bf16 accumulation; the per-element rounding error washes
            # out in the 128-row partition reduction that follows
            if kc == 1:
                nc.vector.tensor_tensor(self.esum[:], self.e[0][:], e[:], ADD)
            elif kc in (2, 3, 6, 7):
                nc.vector.tensor_tensor(self.esum[:], self.esum[:], e[:], ADD)
            elif kc == 5:
                nc.gpsimd.tensor_tensor(self.esg[:], self.e[4][:], e[:], ADD)
            self.e[kc] = e

        def av(self, kc):
            # two K=128 col-tiled matmuls (one per head) run
            # concurrently; head b lands on partitions 64:128 of the
            # single accumulator bank (= final concat layout).
            # start=True clears the whole bank, so h=1's kc=0 matmul
            # lands on cleared has_written bits -> fresh overwrite.
            j = self.j
            e = self.e.pop(kc)
            for h in range(2):
                nc.tensor.matmul(
                    self.avp[h * 64 : (h + 1) * 64, :],
                    v_sb[:, kc, 2 * j + h, :],
                    e[:, h * 512 : (h + 1) * 512],
                    start=(kc == 0),
                    stop=(kc == TC - 1),
                    tile_position=(0, h * 64),
                    skip_group_check=True,
                )

        def dps_drow(self):
            # denominators: partition-reduce esum via K=128 ones
            # matmuls; the reciprocal rides the PSUM evacuation (drow
            # holds 1/denominator).  dps tiles borrow sc-ring slots;
            # they are emitted BEFORE the slot's own score matmul so
            # they land on banks whose ACTIVATEs already retired
            # (otherwise the PE queue serializes behind the scalar
            # engine for a full exp tile).
            j, qc = self.j, self.qc
            self.drow = dr_pool.tile(
                [1, 1024], F32, tag="dr", name=f"dr_{j}_{qc}"
            )
            dps = sc_ps.tile([1, 1024], F32, tag="sc", name=f"dps_{j}_{qc}")
            for h in range(2):
                hsl = slice(h * 512, (h + 1) * 512)
                nc.tensor.matmul(
                    dps[0:1, hsl], cB_sb[:, 0:1], self.esum[:, hsl],
                    start=True, stop=False,
                )
                nc.tensor.matmul(
                    dps[0:1, hsl], cB_sb[:, 0:1], self.esg[:, hsl],
                    start=False, stop=True,
                )
            nc.vector.reciprocal_approx_fast(out=self.drow[:], in_=dps[:])

        def norm_dma(self):
            # broadcast 1/denominator over all 128 partitions via a
            # DRAM bounce (gpsimd queue; lands well before norm_muls)
            j, qc = self.j, self.qc
            rd = rd_pool.tile([1, 1024], F32, tag="rd", name=f"rd_{j}_{qc}")
            nc.gpsimd.dma_start(rd[:], self.drow[:])
            self.rbw = rbw_pool.tile(
                [128, 1024], F32, tag="rbw", name=f"rbw_{j}_{qc}"
            )
            nc.gpsimd.dma_start(self.rbw[:], rd.to_broadcast((128, 1024)))

        def norm_bcast_pe(self):
            # flush path: broadcast 1/denominator via K=1 ones-matmul
            # (no DRAM-bounce latency at the end of the program); the
            # DVE copy stages it in SBUF since the normalize multiply
            # may read only one PSUM operand
            j, qc = self.j, self.qc
            rbsb = rbw_pool.tile(
                [128, 1024], F32, tag="rbsb", name=f"rbsb_{j}_{qc}"
            )
            drow_r = dr_pool.tile(
                [1, 1024], F32R, tag="drr", name=f"drr_{j}_{qc}"
            )
            nc.vector.tensor_copy(out=drow_r[:], in_=self.drow[:])
            for h in range(2):
                rbp = sc_ps.tile(
                    [128, 512], F32, tag="sc", name=f"rbp_{j}_{qc}_{h}"
                )
                nc.tensor.matmul(
                    rbp, cF_sb[0:1, 20:148],
                    drow_r[0:1, h * 512 : (h + 1) * 512],
                    start=True, stop=True,
                )
                nc.vector.tensor_copy(
                    out=rbsb[:, h * 512 : (h + 1) * 512], in_=rbp[:]
                )
            self.rbsb = rbsb

        def norm_muls(self, pe=False):
            # normalize straight out of the AV accumulator bank (one
            # PSUM operand per DVE op: avp is the PSUM side)
            j = self.j
            if pe:
                rb0 = self.rbsb[0:64, 0:512]
                rb1 = self.rbsb[64:128, 512:1024]
            else:
                rb0 = self.rbw[0:64, 0:512]
                rb1 = self.rbw[64:128, 512:1024]
            nc.vector.tensor_mul(
                out=qk_sb[0:64, j, 0, self.qsl],
                in0=self.avp[0:64, :],
                in1=rb0,
            )
            nc.vector.tensor_mul(
                out=qk_sb[64:128, j, 0, self.qsl],
                in0=self.avp[64:128, :],
                in1=rb1,
            )

    # ---- schedule ----------------------------------------------------
    # prologue: just the pair-0 q/k projections -- everything else
    # rides as unit-0 fill so the first exp fires as early as possible
    for step in qk_chunk_steps(0, 0, 0, ring=0):
        step()
    for step in qk_chunk_steps(0, 1, 6, ring=1):
        step()

    # unit 0's fill: v chunks 2-7 then pair-1 projections.  qc=0 units
    # j>=1 fill with pair j+1's projections; qc=1 units fill with the
    # first-half output projection.
    # pair-1's projections are interleaved between v chunks so their
    # bias evacuations retire well before the unit-0/1 boundary
    unit0_fill = deque()
    unit0_fill.extend(v_chunk_steps(0, ring=0))
    unit0_fill.extend(v_chunk_steps(1, ring=0))
    unit0_fill.extend(v_chunk_steps(2, ring=0))
    # pair-1's projections go mid-fill so unit 1's scores aren't gated
    # on the tail of unit 0's fill; later v chunks still beat their
    # AV deadlines (v[c] needed by slot c+2)
    unit0_fill.extend(qk_chunk_steps(1, 0, 1))
    unit0_fill.extend(qk_chunk_steps(1, 1, 7))
    unit0_fill.extend(v_chunk_steps(3, ring=0))
    unit0_fill.extend(v_chunk_steps(4, ring=0))
    unit0_fill.extend(v_chunk_steps(5, ring=0))
    unit0_fill.extend(v_chunk_steps(6, ring=0))
    unit0_fill.extend(v_chunk_steps(7, ring=0))

    prev = None
    gfill = None
    gslots = [NPAIR * TC]
    for qc in range(QN):
        for j in range(NPAIR):
            u = AttnUnit(j, qc)
            cross = qc == 1 and j == 0
            if cross:
                gfill = deque()
                for tcc in range(TC // 2):
                    gfill.extend(proj_chunk_steps(tcc, ring=0, yq=nc.gpsimd))
                gslots = [NPAIR * TC]
            if qc == 0:
                if j == 0:
                    fill, cap = unit0_fill, 12
                elif j == 1:
                    # wp arrives mid-flight: needed from qc=1, and
                    # loading it at t=0 would steal HBM bandwidth
                    nc.sync.dma_start(
                        wp_sb[:], wp_d.rearrange("p (ko n) -> p ko n", ko=KO)
                    )
                    fill = deque(
                        qk_chunk_steps(j + 1, 0, j + 1)
                        + qk_chunk_steps(j + 1, 1, 7 + j)
                    )
                    cap = 3
                elif j < NPAIR - 1:
                    fill = deque(
                        qk_chunk_steps(j + 1, 0, j + 1)
                        + qk_chunk_steps(j + 1, 1, 7 + j)
                    )
                    cap = 3
                else:
                    fill, cap = deque(), 3
            else:
                fill, cap = gfill, 2
            # fixed slots for the previous unit's tail; the qc=0->1
            # crossing runs them earlier so the qc=1 projection fill
            # (which reads pair 5's normalized output) isn't blocked
            sched = (
                {0: "av6", 1: "av7", 2: "dps", 3: "dma", 5: "muls"}
                if cross
                else {0: "av6", 1: "av7", 2: "dps", 4: "dma", 7: "muls"}
            )
            for kc in range(TC):
                # previous unit's PE-side tail events go ahead of this
                # slot's score matmul: av6/av7 are immediately
                # runnable, and dps must grab sc-ring banks whose
                # ACTIVATE retired.  The DVE-side events (muls) go
                # AFTER esum so they don't delay the esum chain in the
                # strict-FIFO DVE queue.
                ev = sched.get(kc) if prev is not None else None
                if ev == "av6":
                    prev.av(6)
                elif ev == "av7":
                    prev.av(7)
                elif ev == "dps":
                    prev.dps_drow()
                u.sc_act(kc)
                if ev == "dma":
                    prev.norm_dma()
                elif ev == "muls":
                    prev.norm_muls()
                if kc >= 2:
                    u.av(kc - 2)
                if fill:
                    if qc == 0:
                        denom = TC - kc
                        n = min(cap, max(1, -(-len(fill) // denom)))
                    else:
                        # ~1 step per slot while the backlog is deep,
                        # every other slot once it thins, leaving a few
                        # for the final-flush drain so the PE never
                        # idles long enough to cool
                        n = (
                            1
                            if (len(fill) > 14 or gslots[0] % 2 == 0)
                            else 0
                        )
                    for _ in range(n):
                        if fill:
                            fill.popleft()()
                gslots[0] -= 1
            if qc == 0:
                while fill:
                    fill.popleft()()
            prev = u

    # ---- final unit's tail + second-half output projection -----------
    # interleave the 4 trailing proj chunks ko-wise (2 on the acc ring,
    # 2 on the freed sc ring) so the last normalization's DRAM bounce
    # hides under matmul work; the ko=5 terms (which read pair 5's
    # normalized output) come after norm_muls.
    prev.av(6)
    prev.av(7)
    prev.dps_drow()
    # reserved gfill steps drain here: they execute while the flush's
    # reciprocal / f32r rounding retire on the DVE, keeping the PE warm
    while gfill:
        gfill.popleft()()
    prev.norm_bcast_pe()
    prev.norm_muls(pe=True)
    tail_q = [nc.sync, nc.gpsimd, nc.scalar, nc.sync]
    tails = [
        proj_chunk_steps(tcc, ring=(0 if tcc < 6 else 1), yq=tail_q[tcc - 4])
        for tcc in range(TC // 2, TC)
    ]
    for ko in range(KO):
        for t in tails:
            t[ko]()
    for t in tails:
        t[KO]()


def _get_program():
    if "nc" in _cache:
        return _cache["nc"]
    nc = bacc.Bacc(
        "TRN2", target_bir_lowering=False, debug=False, enable_asserts=True
    )
    aps = {
        "xT": nc.dram_tensor("xT", [128, KO * T], BF16, kind="ExternalInput").ap(),
        "wv": nc.dram_tensor("wv", [128, KO * C], BF16, kind="ExternalInput").ap(),
        "wqk": nc.dram_tensor(
            "wqk", [128, 12 * KO * 128], BF16, kind="ExternalInput"
        ).ap(),
        "wp": nc.dram_tensor("wp", [128, KO * C], BF16, kind="ExternalInput").ap(),
        "cF": nc.dram_tensor("cF", [128, 148], F32R, kind="ExternalInput").ap(),
        "cB": nc.dram_tensor("cB", [128, 1664], BF16, kind="ExternalInput").ap(),
        "y": nc.dram_tensor("y", [T, C], BF16, kind="ExternalOutput").ap(),
    }
    with tile.TileContext(nc) as tc_ctx, ExitStack() as ctx:
        aps["ctx"] = ctx
        _emit_kernel(tc_ctx, aps)
    nc.compile()
    _cache["nc"] = nc
    return nc


def _p_major(a, ko=KO):
    # [(ko p), n] -> [p, (ko n)] partition-major layout
    n = a.shape[1]
    return np.ascontiguousarray(
        a.reshape(ko, 128, n).transpose(1, 0, 2).reshape(128, ko * n)
    )


def _make_in_maps(inputs):
    x = np.asarray(inputs["x"], np.float32)
    mask = np.asarray(inputs["attn_mask"])
    Wa = np.asarray(inputs["W_attn"], np.float32)
    ba = np.asarray(inputs["b_attn"], np.float32)
    Wp = np.asarray(inputs["W_proj"], np.float32)
    bp = np.asarray(inputs["b_proj"], np.float32)

    wv = _p_major(Wa[:, 2 * C :]).astype(BF)
    wqk = np.concatenate(
        [_p_major(Wa[:, m * 128 : (m + 1) * 128]) for m in range(12)], axis=1
    ).astype(BF)
    wp = _p_major(Wp).astype(BF)

    cB = np.zeros((128, 1664), BF)
    cB[:, 0:128] = 1
    # b_v folds into the projection bias (sum of attention weights is
    # 1, so out_h picks up exactly one b_v): bp' = bp + b_v @ W_proj
    bp_eff = bp + ba[2 * C :] @ Wp
    cB[:, 896:1664] = bp_eff.astype(BF)[None, :]

    cF0 = np.zeros((128, 148), np.float32)
    cF0[:, 0:12] = ba[: 2 * C].reshape(12, 128).T
    cF0[0, 20:148] = 1.0
    in_maps = []
    for b in range(B):
        cF = cF0.copy()
        mb = np.where(mask[b] == 0, np.float32(-30.0), np.float32(0.0))
        cF[:, 12:20] = mb.reshape(TC, 128).T
        in_maps.append(
            {
                "xT": _p_major(np.ascontiguousarray(x[b].T)).astype(BF),
                "wv": wv,
                "wqk": wqk,
                "wp": wp,
                "cF": cF,
                "cB": cB,
            }
        )
    return in_maps


def _run(inputs, trace=False):
    nc = _get_program()
    in_maps = _make_in_maps(inputs)
    res = bass_utils.run_bass_kernel_spmd(
        nc, in_maps, core_ids=list(range(B)), trace=trace
    )
    y = np.stack(
        [res.results[b]["y"].astype(np.float32) for b in range(B)], axis=0
    )
    return y, res


def kernel(**inputs) -> np.ndarray:
    y, _ = _run(inputs, trace=False)
    return y


# revision 54
# speedup vs baseline: 1.0392x; 1.0079x over previous
"""Bass/Trainium2 kernel for a 12-head self-attention block
(B=8, T=1024, C=768), data-parallel across 8 NeuronCores (one batch
element per core).

Per-core computation (batch element b):
  qkv   = x @ W_attn + b_attn            [T, 3C]
  scoresT[k, q] = k_h . q_h / 8 (+ mask bias), keys on partitions
  e     = exp(scoresT)                   (unnormalized)
  out_h = (v_h.T @ e_h) / (sum_k e_h)
  y     = concat(out_h) @ W_proj + b_proj

v6 design (all matmul operands bf16, fp32 PSUM accumulation):
  - attention is a uniform 12-unit (6 head-pairs x 2 query-halves)
    software pipeline; every kc slot issues one score matmul pair
    (row-tiled, concurrent), one AV matmul pair (col-tiled K=128 into
    a single accumulator bank = final concat layout) and ~2
    projection-fill steps, pacing the scalar engine's exp (~1.1us per
    [128,1024] tile)
  - each unit's tail (last two AV groups, denominator reduce via
    K=128 ones-matmuls over a DVE/GpSimd-split esum, reciprocal
    riding the PSUM evacuation, DRAM-bounce broadcast, normalize
    multiplies straight out of the AV bank) is deferred into fixed
    slots of the NEXT unit so no engine stalls at a unit boundary
  - ~16 junk matmuls at t=0 release the HAM clock throttle before the
    real prologue; xT spreads across three DMA queues; the pair-0
    q/k projections are the only serial prologue -- all v chunks and
    pair-1's projections ride as fill inside unit 0
  - b_v folds into b_proj host-side (sum of attention weights is 1),
    so v evacuation is a pure copy on the otherwise-idle scalar
    queue; qk bias evacuation stays on DVE
  - output projection of the first token half fills the qc=1 units;
    the trailing four chunks run after the final flush, whose
    broadcast uses a K=1 ones-matmul instead of the DRAM bounce
"""

import sys

if "/opt/trn_rl_repo" not in sys.path:
    sys.path.insert(0, "/opt/trn_rl_repo")

from collections import deque
from contextlib import ExitStack

import ml_dtypes
import numpy as np

import concourse.bass as bass
import concourse.tile as tile
from concourse import bacc, mybir
from concourse import bass_utils

N_HEAD = 12
B = 8
T = 1024
C = 768
HD = 64
KO = C // 128          # 6 contraction chunks of 128
TC = T // 128          # 8 token chunks of 128
QN = T // 512          # 2 query chunks of 512
NPAIR = N_HEAD // 2    # 6 head pairs

F32 = mybir.dt.float32
F32R = mybir.dt.float32r
BF16 = mybir.dt.bfloat16
AF = mybir.ActivationFunctionType
ADD = mybir.AluOpType.add

_cache: dict = {}
BF = ml_dtypes.bfloat16


def _emit_kernel(tc_ctx, aps):
    nc = tc_ctx.nc
    ctx = aps["ctx"]
    xT_d, wv_d, wqk_d, wp_d, cF_d, cB_d, y_d = (
        aps["xT"], aps["wv"], aps["wqk"], aps["wp"], aps["cF"], aps["cB"],
        aps["y"],
    )

    const = ctx.enter_context(tc_ctx.tile_pool(name="const", bufs=1))
    # 7-deep e ring: when unit 0/1's AVs wait on late v chunks, a
    # shallow ring would stall the ACTIVATE on output-tile allocation
    e_pool = ctx.enter_context(tc_ctx.tile_pool(name="e", bufs=7))
    es_pool = ctx.enter_context(tc_ctx.tile_pool(name="es", bufs=2))
    dr_pool = ctx.enter_context(tc_ctx.tile_pool(name="dr", bufs=2))
    rbw_pool = ctx.enter_context(tc_ctx.tile_pool(name="rbw", bufs=2))
    rd_pool = ctx.enter_context(tc_ctx.tile_pool(name="rd", bufs=2, space="DRAM"))
    out_pool = ctx.enter_context(tc_ctx.tile_pool(name="out", bufs=2))

    # PSUM: 8 banks = scores 2x[128,1024] (4; the ring also lends
    # slots to the per-unit denominator tiles) + AV accumulators (2) +
    # qkv/proj fill accumulators (2)
    sc_ps = ctx.enter_context(tc_ctx.tile_pool(name="scps", bufs=2, space="PSUM"))
    av_ps = ctx.enter_context(tc_ctx.tile_pool(name="avps", bufs=2, space="PSUM"))
    acc_ps = ctx.enter_context(tc_ctx.tile_pool(name="accps", bufs=2, space="PSUM"))

    # ---- persistent SBUF tensors -------------------------------------
    xT_sb = const.tile([128, KO, T], BF16)
    wv_sb = const.tile([128, KO, C], BF16)
    wqk_sb = const.tile([128, 12, KO, 128], BF16)
    wp_sb = const.tile([128, KO, C], BF16)
    qk_sb = const.tile([128, KO, 2, T], BF16)   # [pair, half(q/k), t]
    v_sb = const.tile([128, TC, N_HEAD, HD], BF16)
    # cF: [:,0:12]=bqk | [:,12:20]=mb | [0,20:148]=ones (f32r row for
    # the final flush's K=1 broadcast matmul)
    cF_sb = const.tile([128, 148], F32R)
    # cB: [:,0:128]=ones | [:,128:896]=b_v | [:,896:1664]=b_proj
    # (biases replicated across partitions so DVE adds read lane-local)
    cB_sb = const.tile([128, 1664], BF16)

    # ---- PE pre-warm --------------------------------------------------
    # ~16 junk matmuls release the HAM clock throttle (K=4/8 -> 8/8,
    # ~3.4us of sustained activity) and keep the PE busy through the
    # input-DMA window, so the real prologue matmuls run at 2.4 GHz
    junkw = const.tile([128, 640], BF16)
    junkp = sc_ps.tile([128, 512], F32, tag="sc", name="junkp")
    nc.gpsimd.memset(junkw[:], 0)
    for _ in range(16):
        nc.tensor.matmul(
            junkp[:, 0:256], junkw[:, 0:128], junkw[:, 128:384],
            start=True, stop=True,
        )

    # ---- input DMAs, spread across queues ----------------------------
    # priority: xT + pair-0 wqk feed the first matmuls; wv feeds the
    # v chunks from ~7us; the rest of wqk is needed from unit 0's fill
    nc.gpsimd.dma_start(cF_sb[:], cF_d)
    nc.gpsimd.dma_start(cB_sb[:], cB_d)
    xT_r = xT_d.rearrange("p (ko t) -> p ko t", ko=KO)
    wv_r = wv_d.rearrange("p (ko n) -> p ko n", ko=KO)
    wqk_r = wqk_d.rearrange("p (m ko n) -> p m ko n", m=12, ko=KO)
    # xT spreads over three queues so the prologue isn't serialized
    # behind one queue's ~1.2us-per-chunk service rate
    nc.scalar.dma_start(wqk_sb[:, 0], wqk_r[:, 0])
    nc.scalar.dma_start(wqk_sb[:, 6], wqk_r[:, 6])
    xT_q = [nc.sync, nc.scalar, nc.gpsimd]
    for ko in range(KO):
        xT_q[ko % 3].dma_start(xT_sb[:, ko], xT_r[:, ko])
    for ko in range(KO):
        nc.sync.dma_start(wv_sb[:, ko], wv_r[:, ko])
    for j in range(1, NPAIR):
        nc.gpsimd.dma_start(wqk_sb[:, j], wqk_r[:, j])
        nc.gpsimd.dma_start(wqk_sb[:, 6 + j], wqk_r[:, 6 + j])

    segs = [(0, 512), (512, 256)]
    bqk_sb = cF_sb[:, 0:12]
    mb_sb = cF_sb[:, 12:20]

    def psum_pair(name, ring):
        # a (512, 256)-wide accumulator pair from either psum ring
        if ring == 0:
            return [
                acc_ps.tile([128, 512], F32, tag="acc", name=f"{name}_{i}")
                for i in range(2)
            ]
        big = sc_ps.tile([128, 1024], F32, tag="sc", name=name)
        return [big[:, 0:512], big[:, 512:1024]]

    # ---- phase 1b: v[t, c'] for c' in [1536, 2304) -------------------
    def v_chunk_steps(tcc, ring=0):
        pss = psum_pair(f"ps1b_{tcc}", ring)

        def ko_step(ko):
            def f():
                for i, (off, w) in enumerate(segs):
                    nc.tensor.matmul(
                        pss[i][:, :w],
                        xT_sb[:, ko, tcc * 128 : (tcc + 1) * 128],
                        wv_sb[:, ko, off : off + w],
                        start=(ko == 0),
                        stop=(ko == KO - 1),
                    )
            return f

        def out_step():
            # pure copy: out_h = sum_k a_k (v_k + b_v) = (sum a_k v_k)
            # + b_v since sum a = 1, so b_v folds into b_proj host-side
            for i, (off, w) in enumerate(segs):
                nc.scalar.copy(
                    out=v_sb[:, tcc, off // HD : (off + w) // HD, :],
                    in_=pss[i][:, :w].rearrange("p (h d) -> p h d", d=HD),
                )

        return [ko_step(ko) for ko in range(KO)] + [out_step]

    # ---- phase 1a: qkT chunk m (fill-steppable) ----------------------
    def qk_chunk_steps(j, half, m, ring=0):
        # half 0 -> qT chunk (m = j), half 1 -> kT chunk (m = 6 + j)
        pss = psum_pair(f"ps1a_{m}", ring)

        def ko_step(ko):
            def f():
                for nq in range(QN):
                    nc.tensor.matmul(
                        pss[nq],
                        wqk_sb[:, m, ko, :],
                        xT_sb[:, ko, nq * 512 : (nq + 1) * 512],
                        start=(ko == 0),
                        stop=(ko == KO - 1),
                    )
            return f

        def bias_step():
            # stays on DVE: routing this through the scalar queue
            # would park the next unit's score inputs behind a full
            # unit of exp tiles in the scalar FIFO
            for nq in range(QN):
                nc.vector.tensor_tensor(
                    qk_sb[:, j, half, nq * 512 : (nq + 1) * 512],
                    pss[nq],
                    bqk_sb[:, m : m + 1].to_broadcast((128, 512)),
                    ADD,
                )

        return [ko_step(ko) for ko in range(KO)] + [bias_step]

    # ---- phase 4: one token chunk of y = concatT.T @ W_proj ----------
    def proj_chunk_steps(tcc, ring=0, yq=None):
        pss = psum_pair(f"ps4_{tcc}", ring)
        o_sb = out_pool.tile([128, C], BF16, tag="out", name=f"o_{tcc}")

        def ko_step(ko):
            def f():
                for i, (off, w) in enumerate(segs):
                    nc.tensor.matmul(
                        pss[i][:, :w],
                        qk_sb[:, ko, 0, tcc * 128 : (tcc + 1) * 128],
                        wp_sb[:, ko, off : off + w],
                        start=(ko == 0),
                        stop=(ko == KO - 1),
                    )
            return f

        def out_step():
            for i, (off, w) in enumerate(segs):
                nc.vector.tensor_tensor(
                    o_sb[:, off : off + w],
                    pss[i][:, :w],
                    cB_sb[:, 896 + off : 896 + off + w],
                    ADD,
                )
            (yq or nc.sync).dma_start(y_d[tcc * 128 : (tcc + 1) * 128, :], o_sb[:])

        return [ko_step(ko) for ko in range(KO)] + [out_step]

    # ---- attention unit: one (head pair, query half) -----------------
    class AttnUnit:
        def __init__(self, j, qc):
            self.j, self.qc = j, qc
            self.qsl = slice(qc * 512, (qc + 1) * 512)
            self.avp = av_ps.tile(
                [128, 512], F32, tag="av", name=f"avp_{j}_{qc}"
            )
            # esum splits into a 6-term DVE chain and one independent
            # GpSimd partial (e4+e5) to shave the near-saturated DVE;
            # the denominator matmuls accumulate both partials
            self.esum = es_pool.tile(
                [128, 1024], BF16, tag="es", name=f"es_{j}_{qc}"
            )
            self.esg = es_pool.tile(
                [128, 1024], BF16, tag="esg", name=f"esg_{j}_{qc}"
            )
            self.e = {}
            self.drow = None
            self.rbw = None

        def sc_act(self, kc):
            j, qc = self.j, self.qc
            ksl = slice(kc * 128, (kc + 1) * 128)
            sc = sc_ps.tile(
                [128, 1024], F32, tag="sc", name=f"sc_{j}_{qc}_{kc}"
            )
            nc.tensor.matmul(
                sc[:, 0:512], qk_sb[0:64, j, 1, ksl], qk_sb[0:64, j, 0, self.qsl],
                start=True, stop=True, tile_position=(0, 0),
            )
            nc.tensor.matmul(
                sc[:, 512:1024], qk_sb[64:128, j, 1, ksl],
                qk_sb[64:128, j, 0, self.qsl],
                start=True, stop=True, tile_position=(64, 0),
            )
            e = e_pool.tile([128, 1024], BF16, tag="e", name=f"e_{j}_{qc}_{kc}")
            nc.scalar.activation(
                e, sc, AF.Exp, bias=mb_sb[:, kc : kc + 1], scale=0.125
            )
            # bf16 accumulation; the per-element rounding error washes
            # out in the 128-row partition reduction that follows
            if kc == 1:
                nc.vector.tensor_tensor(self.esum[:], self.e[0][:], e[:], ADD)
            elif kc in (2, 3, 6, 7):
                nc.vector.tensor_tensor(self.esum[:], self.esum[:], e[:], ADD)
            elif kc == 5:
                nc.gpsimd.tensor_tensor(self.esg[:], self.e[4][:], e[:], ADD)
            self.e[kc] = e

        def av(self, kc):
            # two K=128 col-tiled matmuls (one per head) run
            # concurrently; head b lands on partitions 64:128 of the
            # single accumulator bank (= final concat layout).
            # start=True clears the whole bank, so h=1's kc=0 matmul
            # lands on cleared has_written bits -> fresh overwrite.
            j = self.j
            e = self.e.pop(kc)
            for h in range(2):
                nc.tensor.matmul(
                    self.avp[h * 64 : (h + 1) * 64, :],
                    v_sb[:, kc, 2 * j + h, :],
                    e[:, h * 512 : (h + 1) * 512],
                    start=(kc == 0),
                    stop=(kc == TC - 1),
                    tile_position=(0, h * 64),
                    skip_group_check=True,
                )

        def dps_drow(self):
            # denominators: partition-reduce esum via K=128 ones
            # matmuls; the reciprocal rides the PSUM evacuation (drow
            # holds 1/denominator).  dps tiles borrow sc-ring slots;
            # they are emitted BEFORE the slot's own score matmul so
            # they land on banks whose ACTIVATEs already retired
            # (otherwise the PE queue serializes behind the scalar
            # engine for a full exp tile).
            j, qc = self.j, self.qc
            self.drow = dr_pool.tile(
                [1, 1024], F32, tag="dr", name=f"dr_{j}_{qc}"
            )
            dps = sc_ps.tile([1, 1024], F32, tag="sc", name=f"dps_{j}_{qc}")
            for h in range(2):
                hsl = slice(h * 512, (h + 1) * 512)
                nc.tensor.matmul(
                    dps[0:1, hsl], cB_sb[:, 0:1], self.esum[:, hsl],
                    start=True, stop=False,
                )
                nc.tensor.matmul(
                    dps[0:1, hsl], cB_sb[:, 0:1], self.esg[:, hsl],
                    start=False, stop=True,
                )
            nc.vector.reciprocal_approx_fast(out=self.drow[:], in_=dps[:])

        def norm_dma(self):
            # broadcast 1/denominator over all 128 partitions via a
            # DRAM bounce (gpsimd queue; lands well before norm_muls)
            j, qc = self.j, self.qc
            rd = rd_pool.tile([1, 1024], F32, tag="rd", name=f"rd_{j}_{qc}")
            nc.gpsimd.dma_start(rd[:], self.drow[:])
            self.rbw = rbw_pool.tile(
                [128, 1024], F32, tag="rbw", name=f"rbw_{j}_{qc}"
            )
            nc.gpsimd.dma_start(self.rbw[:], rd.to_broadcast((128, 1024)))

        def norm_bcast_pe(self):
            # flush path: broadcast 1/denominator via K=1 ones-matmul
            # (no DRAM-bounce latency at the end of the program); the
            # DVE copy stages it in SBUF since the normalize multiply
            # may read only one PSUM operand
            j, qc = self.j, self.qc
            rbsb = rbw_pool.tile(
                [128, 1024], F32, tag="rbsb", name=f"rbsb_{j}_{qc}"
            )
            drow_r = dr_pool.tile(
                [1, 1024], F32R, tag="drr", name=f"drr_{j}_{qc}"
            )
            nc.vector.tensor_copy(out=drow_r[:], in_=self.drow[:])
            for h in range(2):
                rbp = sc_ps.tile(
                    [128, 512], F32, tag="sc", name=f"rbp_{j}_{qc}_{h}"
                )
                nc.tensor.matmul(
                    rbp, cF_sb[0:1, 20:148],
                    drow_r[0:1, h * 512 : (h + 1) * 512],
                    start=True, stop=True,
                )
                nc.vector.tensor_copy(
                    out=rbsb[:, h * 512 : (h + 1) * 512], in_=rbp[:]
                )
            self.rbsb = rbsb

        def norm_muls(self, pe=False):
            # normalize straight out of the AV accumulator bank (one
            # PSUM operand per DVE op: avp is the PSUM side)
            j = self.j
            if pe:
                rb0 = self.rbsb[0:64, 0:512]
                rb1 = self.rbsb[64:128, 512:1024]
            else:
                rb0 = self.rbw[0:64, 0:512]
                rb1 = self.rbw[64:128, 512:1024]
            nc.vector.tensor_mul(
                out=qk_sb[0:64, j, 0, self.qsl],
                in0=self.avp[0:64, :],
                in1=rb0,
            )
            nc.vector.tensor_mul(
                out=qk_sb[64:128, j, 0, self.qsl],
                in0=self.avp[64:128, :],
                in1=rb1,
            )

    # ---- schedule ----------------------------------------------------
    # prologue: just the pair-0 q/k projections -- everything else
    # rides as unit-0 fill so the first exp fires as early as possible
    for step in qk_chunk_steps(0, 0, 0, ring=0):
        step()
    for step in qk_chunk_steps(0, 1, 6, ring=1):
        step()

    # unit 0's fill: v chunks 2-7 then pair-1 projections.  qc=0 units
    # j>=1 fill with pair j+1's projections; qc=1 units fill with the
    # first-half output projection.
    # pair-1's projections are interleaved between v chunks so their
    # bias evacuations retire well before the unit-0/1 boundary
    unit0_fill = deque()
    unit0_fill.extend(v_chunk_steps(0, ring=0))
    unit0_fill.extend(v_chunk_steps(1, ring=0))
    unit0_fill.extend(v_chunk_steps(2, ring=0))
    # pair-1's projections go mid-fill so unit 1's scores aren't gated
    # on the tail of unit 0's fill; later v chunks still beat their
    # AV deadlines (v[c] needed by slot c+2)
    unit0_fill.extend(qk_chunk_steps(1, 0, 1))
    unit0_fill.extend(qk_chunk_steps(1, 1, 7))
    unit0_fill.extend(v_chunk_steps(3, ring=0))
    unit0_fill.extend(v_chunk_steps(4, ring=0))
    unit0_fill.extend(v_chunk_steps(5, ring=0))
    unit0_fill.extend(v_chunk_steps(6, ring=0))
    unit0_fill.extend(v_chunk_steps(7, ring=0))

    prev = None
    gfill = None
    gslots = [NPAIR * TC]
    for qc in range(QN):
        for j in range(NPAIR):
            u = AttnUnit(j, qc)
            cross = qc == 1 and j == 0
            if cross:
                gfill = deque()
                for tcc in range(TC // 2):
                    gfill.extend(proj_chunk_steps(tcc, ring=0, yq=nc.gpsimd))
                gslots = [NPAIR * TC]
            if qc == 0:
                if j == 0:
                    fill, cap = unit0_fill, 12
                elif j == 1:
                    # wp arrives mid-flight: needed from qc=1, and
                    # loading it at t=0 would steal HBM bandwidth
                    nc.sync.dma_start(
                        wp_sb[:], wp_d.rearrange("p (ko n) -> p ko n", ko=KO)
                    )
                    fill = deque(
                        qk_chunk_steps(j + 1, 0, j + 1)
                        + qk_chunk_steps(j + 1, 1, 7 + j)
                    )
                    cap = 3
                elif j < NPAIR - 1:
                    fill = deque(
                        qk_chunk_steps(j + 1, 0, j + 1)
                        + qk_chunk_steps(j + 1, 1, 7 + j)
                    )
                    cap = 3
                else:
                    fill, cap = deque(), 3
            else:
                fill, cap = gfill, 2
            # fixed slots for the previous unit's tail; the qc=0->1
            # crossing runs them earlier so the qc=1 projection fill
            # (which reads pair 5's normalized output) isn't blocked
            sched = (
                {0: "av6", 1: "av7", 2: "dps", 3: "dma", 5: "muls"}
                if cross
                else {0: "av6", 1: "av7", 2: "dps", 4: "dma", 7: "muls"}
            )
            for kc in range(TC):
                # previous unit's PE-side tail events go ahead of this
                # slot's score matmul: av6/av7 are immediately
                # runnable, and dps must grab sc-ring banks whose
                # ACTIVATE retired.  The DVE-side events (muls) go
                # AFTER esum so they don't delay the esum chain in the
                # strict-FIFO DVE queue.
                ev = sched.get(kc) if prev is not None else None
                if ev == "av6":
                    prev.av(6)
                elif ev == "av7":
                    prev.av(7)
                elif ev == "dps":
                    prev.dps_drow()
                u.sc_act(kc)
                if ev == "dma":
                    prev.norm_dma()
                elif ev == "muls":
                    prev.norm_muls()
                if kc >= 2:
                    u.av(kc - 2)
                if fill:
                    if qc == 0:
                        denom = TC - kc
                        n = min(cap, max(1, -(-len(fill) // denom)))
                    else:
                        # ~1 step per slot while the backlog is deep,
                        # every other slot once it thins, leaving a few
                        # for the final-flush drain so the PE never
                        # idles long enough to cool
                        n = (
                            1
                            if (len(fill) > 14 or gslots[0] % 2 == 0)
                            else 0
                        )
                    for _ in range(n):
                        if fill:
                            fill.popleft()()
                gslots[0] -= 1
            if qc == 0:
                while fill:
                    fill.popleft()()
            prev = u

    # ---- final unit's tail + second-half output projection -----------
    # interleave the 4 trailing proj chunks ko-wise (2 on the acc ring,
    # 2 on the freed sc ring) so the last normalization's DRAM bounce
    # hides under matmul work; the ko=5 terms (which read pair 5's
    # normalized output) come after norm_muls.
    prev.av(6)
    prev.av(7)
    prev.dps_drow()
    # reserved gfill steps drain here: they execute while the flush's
    # reciprocal / f32r rounding retire on the DVE, keeping the PE warm
    while gfill:
        gfill.popleft()()
    prev.norm_bcast_pe()
    prev.norm_muls(pe=True)
    tail_q = [nc.sync, nc.gpsimd, nc.scalar, nc.sync]
    tails = [
        proj_chunk_steps(tcc, ring=(0 if tcc < 6 else 1), yq=tail_q[tcc - 4])
        for tcc in range(TC // 2, TC)
    ]
    for ko in range(KO):
        for t in tails:
            t[ko]()
    for t in tails:
        t[KO]()


def _get_program():
    if "nc" in _cache:
        return _cache["nc"]
    nc = bacc.Bacc(
        "TRN2", target_bir_lowering=False, debug=False, enable_asserts=True
    )
    aps = {
        "xT": nc.dram_tensor("xT", [128, KO * T], BF16, kind="ExternalInput").ap(),
        "wv": nc.dram_tensor("wv", [128, KO * C], BF16, kind="ExternalInput").ap(),
        "wqk": nc.dram_tensor(
            "wqk", [128, 12 * KO * 128], BF16, kind="ExternalInput"
        ).ap(),
        "wp": nc.dram_tensor("wp", [128, KO * C], BF16, kind="ExternalInput").ap(),
        "cF": nc.dram_tensor("cF", [128, 148], F32R, kind="ExternalInput").ap(),
        "cB": nc.dram_tensor("cB", [128, 1664], BF16, kind="ExternalInput").ap(),
        "y": nc.dram_tensor("y", [T, C], BF16, kind="ExternalOutput").ap(),
    }
    with tile.TileContext(nc) as tc_ctx, ExitStack() as ctx:
        aps["ctx"] = ctx
        _emit_kernel(tc_ctx, aps)
    nc.compile()
    _cache["nc"] = nc
    return nc


def _p_major(a, ko=KO):
    # [(ko p), n] -> [p, (ko n)] partition-major layout
    n = a.shape[1]
    return np.ascontiguousarray(
        a.reshape(ko, 128, n).transpose(1, 0, 2).reshape(128, ko * n)
    )


def _make_in_maps(inputs):
    x = np.asarray(inputs["x"], np.float32)
    mask = np.asarray(inputs["attn_mask"])
    Wa = np.asarray(inputs["W_attn"], np.float32)
    ba = np.asarray(inputs["b_attn"], np.float32)
    Wp = np.asarray(inputs["W_proj"], np.float32)
    bp = np.asarray(inputs["b_proj"], np.float32)

    wv = _p_major(Wa[:, 2 * C :]).astype(BF)
    wqk = np.concatenate(
        [_p_major(Wa[:, m * 128 : (m + 1) * 128]) for m in range(12)], axis=1
    ).astype(BF)
    wp = _p_major(Wp).astype(BF)

    cB = np.zeros((128, 1664), BF)
    cB[:, 0:128] = 1
    # b_v folds into the projection bias (sum of attention weights is
    # 1, so out_h picks up exactly one b_v): bp' = bp + b_v @ W_proj
    bp_eff = bp + ba[2 * C :] @ Wp
    cB[:, 896:1664] = bp_eff.astype(BF)[None, :]

    cF0 = np.zeros((128, 148), np.float32)
    cF0[:, 0:12] = ba[: 2 * C].reshape(12, 128).T
    cF0[0, 20:148] = 1.0
    in_maps = []
    for b in range(B):
        cF = cF0.copy()
        mb = np.where(mask[b] == 0, np.float32(-30.0), np.float32(0.0))
        cF[:, 12:20] = mb.reshape(TC, 128).T
        in_maps.append(
            {
                "xT": _p_major(np.ascontiguousarray(x[b].T)).astype(BF),
                "wv": wv,
                "wqk": wqk,
                "wp": wp,
                "cF": cF,
                "cB": cB,
            }
        )
    return in_maps


def _run(inputs, trace=False):
    nc = _get_program()
    in_maps = _make_in_maps(inputs)
    res = bass_utils.run_bass_kernel_spmd(
        nc, in_maps, core_ids=list(range(B)), trace=trace
    )
    y = np.stack(
        [res.results[b]["y"].astype(np.float32) for b in range(B)], axis=0
    )
    return y, res


def kernel(**inputs) -> np.ndarray:
    y, _ = _run(inputs, trace=False)
    return y
